# revision 1
# baseline (speedup 1.0000x reference)
"""DenoiseGAT Trainium2 kernel: 8-core data-parallel over polygons (cycle graphs).

Per core: 256 polygons x 64 nodes = 16384 nodes. Activations as h^T
(features x nodes, bf16), 256-row tensors stored as [128, 2, n] tiles
(half index on the free dim). Attention: scores via block-diag a-matmul;
softmax in poly-partition block layout; alpha replicated to feature rows
via DRAM-staged broadcast DMA; neighbor combine via +-1 shifted tensor
ops (shifts stay inside 64-node polygons).
"""

import numpy as np
import ml_dtypes
from contextlib import ExitStack

import concourse.bass as bass
import concourse.tile as tile
import concourse.tile_utils as tile_utils
from concourse import bacc, mybir
from concourse.bass_utils import run_bass_kernel_spmd

tile_utils.max_sbuf_usage = 208 * 1024

F32 = mybir.dt.float32
BF16 = mybir.dt.bfloat16
ALU = mybir.AluOpType
ACTF = mybir.ActivationFunctionType

NCORES = 8
B, V = 2048, 64
HID, TDIM = 256, 128
BC = B // NCORES            # 256 polygons / core
N = BC * V                  # 16384 nodes / core
NT = 512                    # matmul node tile
SCH = 8192                  # softmax chunk = 128 polys
CCH = 1024                  # combine subchunk = 16 polys


def _ablk(asrc, atgt):
    NH, FO = asrc.shape
    out = np.zeros((NH * FO, 2 * NH), np.float32)
    for h in range(NH):
        out[h * FO:(h + 1) * FO, h] = asrc[h]
        out[h * FO:(h + 1) * FO, NH + h] = atgt[h]
    return out


def _bf(a):
    return np.ascontiguousarray(np.asarray(a, np.float32).astype(ml_dtypes.bfloat16))


def _f32(a):
    return np.ascontiguousarray(np.asarray(a, np.float32))


def _poly(ap, v=V):
    return ap.rearrange("p (g v) -> p g v", v=v)


def build(weights):
    nc = bacc.Bacc("TRN2", target_bir_lowering=False, debug=False,
                   enable_asserts=False, num_devices=NCORES)
    w = weights

    def inl(name, arr):
        return nc.inline_tensor(np.ascontiguousarray(arr), name=name).ap()

    half = TDIM // 2
    freqs = np.exp(-np.log(10000.0) * np.arange(half, dtype=np.float32) / (half - 1))
    fr2 = np.stack([np.concatenate([freqs, freqs]),
                    np.concatenate([np.zeros(half, np.float32),
                                    np.full(half, np.pi / 2, np.float32)])])
    ph = np.arange(V, dtype=np.float32) * (2 * np.pi / V)
    posT = np.tile(np.stack([np.sin(ph), np.cos(ph), np.sin(2 * ph), np.cos(2 * ph)]), (1, BC))

    def half3(a):
        """(256, X) host -> (128, 2, X) so tile[:, j, :] == rows 128j:128j+128."""
        a = np.asarray(a)
        return np.ascontiguousarray(a.reshape(2, 128, a.shape[1]).transpose(1, 0, 2))

    W0 = _f32(w["W0"]); sk0 = _f32(w["skip0"]); ab0 = _ablk(_f32(w["asrc0"]), _f32(w["atgt0"]))
    c_fr2 = inl("fr2", fr2.astype(np.float32))
    c_tW = inl("tW", _f32(w["tW"]))
    c_tb = inl("tb", _f32(w["tb"]).reshape(-1, 1))
    c_posT = inl("posT", _bf(posT))
    c_Wsum0t = inl("Wsum0t", W0[6:] + sk0[6:])          # (128, 256)
    c_W0ab = inl("W0ab", W0[6:] @ ab0)                  # (128, 8)
    c_W0f = inl("W0f", _bf(np.concatenate([W0[:6], sk0[:6]], 1)))   # (6, 512)
    c_ab0 = inl("ab0", half3(_bf(ab0)))                 # (128, 2, 8)
    c_b0 = inl("b0c", half3(_f32(w["b0"]).reshape(-1, 1)))
    c_eye8 = inl("eye8", np.eye(8, dtype=np.float32))
    LW, LAB, LB = {}, {}, {}
    for i in (1, 2):
        LW[i] = inl(f"W{i}f", half3(_bf(np.concatenate([_f32(w[f"W{i}"]), _f32(w[f"skip{i}"])], 1))))
        LAB[i] = inl(f"ab{i}f", half3(_bf(_ablk(_f32(w[f"asrc{i}"]), _f32(w[f"atgt{i}"])))))
        LB[i] = inl(f"b{i}c", half3(_f32(w[f"b{i}"]).reshape(-1, 1)))
    c_W3 = inl("W3f", half3(_bf(_f32(w["W3"]))))
    c_ab3 = inl("ab3f", half3(_bf(_ablk(_f32(w["asrc3"]), _f32(w["atgt3"])))))
    c_b3 = inl("b3c", half3(_f32(w["b3"]).reshape(-1, 1)))
    c_h1W = inl("h1Wf", half3(_bf(_f32(w["h1W"]))))
    c_h1b = inl("h1bc", half3(_f32(w["h1b"]).reshape(-1, 1)))
    c_h2W = inl("h2Wf", half3(_bf(_f32(w["h2W"]))))
    c_h2b = inl("h2bc", _f32(w["h2b"]).reshape(-1, 1))

    xT = nc.dram_tensor("xT", [2, N], BF16, kind="ExternalInput").ap()
    tp = nc.dram_tensor("tp", [2, BC], F32, kind="ExternalInput").ap()
    yT = nc.dram_tensor("yT", [2, N], F32, kind="ExternalOutput").ap()

    with tile.TileContext(nc) as tc, ExitStack() as ctx:
        P = ctx.enter_context(tc.tile_pool(name="pers", bufs=1))
        WP = ctx.enter_context(tc.tile_pool(name="wts", bufs=1))
        DR = ctx.enter_context(tc.tile_pool(name="dram", bufs=1, space="DRAM"))
        PS = ctx.enter_context(tc.tile_pool(name="ps", bufs=5, space="PSUM"))
        PSC = ctx.enter_context(tc.tile_pool(name="pssc", bufs=3, space="PSUM"))
        SM = ctx.enter_context(tc.tile_pool(name="sm", bufs=1))
        CB = ctx.enter_context(tc.tile_pool(name="cb", bufs=1))
        SK = ctx.enter_context(tc.tile_pool(name="sk", bufs=2))
        PJ = ctx.enter_context(tc.tile_pool(name="pj", bufs=1))

        h = P.tile([128, 2, N], BF16, tag="h")
        tembT = P.tile([TDIM, BC], F32, tag="tembT")
        G0T = P.tile([128, 2, BC], F32, tag="G0T")
        s0gT2 = P.tile([128, 2, 8], F32, tag="s0gT2")

        def load(c_ap, tag):
            t = WP.tile(list(c_ap.shape), c_ap.dtype, tag=tag)
            nc.sync.dma_start(t[:], c_ap)
            return t

        t_eye8 = load(c_eye8, "eye8")
        t_fr2 = load(c_fr2, "fr2")
        t_tp = load(tp, "tp")
        ps_te = PSC.tile([TDIM, BC], F32, tag="psA")
        nc.tensor.matmul(ps_te[:], t_fr2[:], t_tp[:], start=True, stop=True)
        te_m = SM.tile([TDIM, BC], F32, tag="Sblk")
        te_q = SM.tile([TDIM, BC], mybir.dt.int32, tag="den")
        nc.vector.tensor_scalar(te_q[:], ps_te[:], float(1.0 / (2 * np.pi)), None, op0=ALU.mult)
        te_qf = SM.tile([TDIM, BC], F32, tag="rd")
        nc.vector.tensor_copy(te_qf[:], te_q[:])
        nc.vector.scalar_tensor_tensor(te_m[:], te_qf[:], float(-2 * np.pi), ps_te[:],
                                       op0=ALU.mult, op1=ALU.add)
        te_s = SM.tile([TDIM, BC], F32, tag="E")
        nc.scalar.activation(te_s[:], te_m[:], ACTF.Sin)
        t_tW = load(c_tW, "tW")
        t_tb = load(c_tb, "tb")
        ps_tm = PSC.tile([TDIM, BC], F32, tag="psA")
        nc.tensor.matmul(ps_tm[:], t_tW[:], te_s[:], start=True, stop=True)
        nc.scalar.activation(tembT[:], ps_tm[:], ACTF.Silu, bias=t_tb[:])

        t_Ws0 = load(c_Wsum0t, "Ws0")
        for m in range(2):
            ps_g = PSC.tile([128, BC], F32, tag="psA")
            nc.tensor.matmul(ps_g[:], t_Ws0[:, m * 128:(m + 1) * 128], tembT[:],
                             start=True, stop=True)
            nc.vector.tensor_copy(G0T[:, m, :], ps_g[:])
        t_W0ab = load(c_W0ab, "W0ab")
        ps_sg = PSC.tile([8, BC], F32, tag="psA")
        nc.tensor.matmul(ps_sg[:], t_W0ab[:], tembT[:], start=True, stop=True)
        s0g = SM.tile([8, BC], F32, tag="EX")
        nc.vector.tensor_copy(s0g[:], ps_sg[:])
        for m in range(2):
            ps_t = PSC.tile([128, 8], F32, tag="psA")
            nc.tensor.matmul(ps_t[:], s0g[:, m * 128:(m + 1) * 128], t_eye8[:],
                             is_transpose=True, start=True, stop=True)
            nc.vector.tensor_copy(s0gT2[:, m, :], ps_t[:])

        h0loc = PJ.tile([6, SCH], BF16, tag="h0loc")
        TT = nc.vector.tensor_tensor
        GT = nc.gpsimd.tensor_tensor
        STT = nc.vector.scalar_tensor_tensor

        def layer(li, FIN, R, FO, c_w, c_ab, c_bias, first_layer):
            FOW = R * FO                 # 256
            nmt = (2 * FOW if li < 3 else FOW) // 128   # output 128-blocks
            kt = (FIN + 127) // 128
            t_w = load(c_w, f"w{li}")    # (6,512) L0 else (128, 2, 512|256)
            t_ab = load(c_ab, f"ab{li}")  # (128, 2, 2R)
            t_b = load(c_bias, f"b{li}")  # (128, 2, 1)
            a_dram = DR.tile([3 * R, N], BF16, tag="a_dram")
            sc_dram = DR.tile([2 * R, N], BF16, tag="sc_dram")

            def lhs_w(k, m):
                kk = min(128, FIN - k * 128)
                if first_layer:
                    return t_w[0:kk, m * 128:(m + 1) * 128]
                return t_w[0:kk, k, m * 128:(m + 1) * 128]

            for ch in range(N // SCH):
                u0 = ch * SCH
                if first_layer:
                    nc.sync.dma_start(h0loc[0:2, :], xT[:, u0:u0 + SCH])
                    nc.sync.dma_start(h0loc[2:6, :], c_posT[:, u0:u0 + SCH])
                projc = PJ.tile([128, 2, SCH], BF16, tag="projc")
                skc = None
                if li < 3:
                    skc = PJ.tile([128, 2, SCH], BF16, tag="skc")
                scT = SM.tile([128, SCH // 4], BF16, tag="scT")

                for it in range(SCH // NT):
                    u = it * NT
                    pst = [PS.tile([128, NT], F32, tag="mm", name=f"mm{_m}") for _m in range(nmt)]
                    for m in range(nmt):
                        for k in range(kt):
                            kk = min(128, FIN - k * 128)
                            rhs = (h0loc[0:kk, u:u + NT] if first_layer
                                   else h[0:kk, k, u0 + u:u0 + u + NT])
                            nc.tensor.matmul(pst[m][:], lhs_w(k, m), rhs,
                                             start=(k == 0), stop=(k == kt - 1))
                    for m in range(nmt):
                        if m < FOW // 128:
                            nc.scalar.activation(projc[:, m, u:u + NT], pst[m][:], ACTF.Copy)
                        else:
                            nc.scalar.activation(skc[:, m - 2, u:u + NT], pst[m][:],
                                                 ACTF.Identity, bias=t_b[:, m - 2, :])
                    s = it % 4
                    if s == 0:
                        scp = PSC.tile([128, NT], F32, tag="psA")
                    for k in range(2):
                        nc.tensor.matmul(scp[32 * s:32 * s + 2 * R, :], t_ab[:, k, :],
                                         projc[:, k, u:u + NT], start=(k == 0), stop=(k == 1),
                                         tile_position=(0, 32 * s))
                    if s == 3:
                        g = it // 4
                        nc.scalar.activation(scT[:, g * NT:(g + 1) * NT], scp[:], ACTF.Copy)

                scd = sc_dram[:, u0:u0 + SCH].rearrange("r (cb s w) -> r cb s w", s=4, w=NT)
                for s in range(4):
                    src = scT[32 * s:32 * s + 2 * R, :].rearrange("p (cb w) -> p cb w", w=NT)
                    nc.sync.dma_start(scd[:, :, s, :], src)
                S = SM.tile([128, 2 * R * V], BF16, tag="Sblk")
                src = sc_dram[:, u0:u0 + SCH].rearrange("r (p v) -> p r v", v=V)
                nc.sync.dma_start(S[:].rearrange("p (r v) -> p r v", v=V), src)

                if first_layer:
                    gb = s0gT2[:, ch, :].unsqueeze(2).to_broadcast((128, 2 * R, V))
                    Sv = S[:].rearrange("p (r v) -> p r v", v=V)
                    TT(Sv, Sv, gb, op=ALU.add)

                E = SM.tile([128, 3 * R * V], BF16, tag="E")
                Sv = S[:].rearrange("p (r v) -> p r v", v=V)
                Ssrc, Stgt = Sv[:, 0:R, :], Sv[:, R:2 * R, :]
                Ev = E[:].rearrange("p (k r v) -> p k r v", k=3, v=V)
                TT(Ev[:, 0, :, 1:], Ssrc[:, :, :V - 1], Stgt[:, :, 1:], op=ALU.add)
                TT(Ev[:, 0, :, 0:1], Ssrc[:, :, V - 1:], Stgt[:, :, 0:1], op=ALU.add)
                TT(Ev[:, 1, :, :], Ssrc, Stgt, op=ALU.add)
                TT(Ev[:, 2, :, :V - 1], Ssrc[:, :, 1:], Stgt[:, :, :V - 1], op=ALU.add)
                TT(Ev[:, 2, :, V - 1:], Ssrc[:, :, 0:1], Stgt[:, :, V - 1:], op=ALU.add)
                STT(E[:], E[:], 0.2, E[:], op0=ALU.mult, op1=ALU.max)
                EX = SM.tile([128, 3 * R * V], BF16, tag="EX")
                nc.scalar.activation(EX[:], E[:], ACTF.Exp)
                den = SM.tile([128, R * V], F32, tag="den")
                TT(den[:], EX[:, 0:R * V], EX[:, R * V:2 * R * V], op=ALU.add)
                TT(den[:], den[:], EX[:, 2 * R * V:], op=ALU.add)
                rd = SM.tile([128, R * V], F32, tag="rd")
                nc.vector.reciprocal(rd[:], den[:])
                ab_blk = SM.tile([128, 3 * R * V], BF16, tag="ab_blk")
                for k in range(3):
                    TT(ab_blk[:, k * R * V:(k + 1) * R * V],
                       EX[:, k * R * V:(k + 1) * R * V], rd[:], op=ALU.mult)
                nc.sync.dma_start(
                    a_dram[:, u0:u0 + SCH].rearrange("j (p v) -> p j v", v=V),
                    ab_blk[:].rearrange("p (j v) -> p j v", v=V))

                blk = min(FO, 128)
                for sc in range(SCH // CCH):
                    v0 = sc * CCH
                    span = slice(u0 + v0, u0 + v0 + CCH)
                    af = [CB.tile([128, 2, CCH], BF16, tag=f"af{k}", name=f"af{k}") for k in range(3)]
                    for k in range(3):
                        for b0 in range(0, FOW, blk):
                            hh = b0 // FO
                            src = a_dram[k * R + hh:k * R + hh + 1, span]
                            nc.sync.dma_start(
                                af[k][b0 % 128:b0 % 128 + blk, b0 // 128, :],
                                src.to_broadcast((blk, CCH)))
                    C1 = CB.tile([128, 2, CCH], BF16, tag="C1")
                    C2 = CB.tile([128, 2, CCH], BF16, tag="C2")
                    C4 = CB.tile([128, 2, CCH], BF16, tag="C4")
                    for ht in range(2):
                        pjv = _poly(projc[:, ht, v0:v0 + CCH])
                        a0 = _poly(af[1][:, ht, :]); ap1 = _poly(af[2][:, ht, :])
                        am1 = _poly(af[0][:, ht, :])
                        c1 = _poly(C1[:, ht, :]); c2 = _poly(C2[:, ht, :]); c4 = _poly(C4[:, ht, :])
                        TT(c1, a0, pjv, op=ALU.mult)
                        GT(c2[:, :, :V - 1], ap1[:, :, :V - 1], pjv[:, :, 1:], op=ALU.mult)
                        GT(c2[:, :, V - 1:], ap1[:, :, V - 1:], pjv[:, :, 0:1], op=ALU.mult)
                        TT(c4[:, :, 1:], am1[:, :, 1:], pjv[:, :, :V - 1], op=ALU.mult)
                        TT(c4[:, :, 0:1], am1[:, :, 0:1], pjv[:, :, V - 1:], op=ALU.mult)
                    C3 = CB.tile([128, 2, CCH], BF16, tag="C3")
                    TT(C3[:], C1[:], C4[:], op=ALU.add)
                    pre = CB.tile([128, 2, CCH], BF16, tag="pre")
                    GT(pre[:], C3[:], C2[:], op=ALU.add)
                    if li < 3:
                        GT(pre[:], pre[:], skc[:, :, v0:v0 + CCH], op=ALU.add)
                        if first_layer:
                            g0 = (u0 + v0) // V
                            for ht in range(2):
                                gbh = G0T[:, ht, g0:g0 + CCH // V].unsqueeze(2).to_broadcast(
                                    (128, CCH // V, V))
                                pvh = _poly(pre[:, ht, :])
                                TT(pvh, pvh, gbh, op=ALU.add)
                        mn = CB.tile([128, 2, CCH], BF16, tag="C1")
                        nc.vector.tensor_scalar(mn[:], pre[:], 0.0, None, op0=ALU.min)
                        ex = CB.tile([128, 2, CCH], BF16, tag="C2")
                        nc.scalar.activation(ex[:], mn[:], ACTF.Exp)
                        rl = CB.tile([128, 2, CCH], BF16, tag="C4")
                        nc.vector.tensor_scalar(rl[:], pre[:], 0.0, None, op0=ALU.max)
                        STT(h[:, :, span], ex[:], -1.0, rl[:], op0=ALU.add, op1=ALU.add)
                    else:
                        out3 = CB.tile([128, 2, CCH], BF16, tag="C1")
                        for ht in range(2):
                            STT(out3[:, ht, :], pre[:, ht, :], t_b[:, ht, :],
                                h[:, ht, span], op0=ALU.add, op1=ALU.add)
                        nc.vector.tensor_copy(h[:, :, span], out3[:])

        layer(0, 6, 4, 64, c_W0f, c_ab0, c_b0, True)
        layer(1, 256, 4, 64, LW[1], LAB[1], LB[1], False)
        layer(2, 256, 4, 64, LW[2], LAB[2], LB[2], False)
        layer(3, 256, 1, 256, c_W3, c_ab3, c_b3, False)

        t_h1W = load(c_h1W, "h1W")
        t_h1b = load(c_h1b, "h1b")
        t_h2W = load(c_h2W, "h2W")
        t_h2b = load(c_h2b, "h2b")
        for it in range(N // NT):
            u = it * NT
            pst = [PS.tile([128, NT], F32, tag="mm", name=f"mmh{_m}") for _m in range(2)]
            for m in range(2):
                for k in range(2):
                    nc.tensor.matmul(pst[m][:], t_h1W[:, k, m * 128:(m + 1) * 128],
                                     h[:, k, u:u + NT], start=(k == 0), stop=(k == 1))
            h5 = CB.tile([128, 2, NT], BF16, tag="h5")
            for m in range(2):
                nc.scalar.activation(h5[:, m, :], pst[m][:], ACTF.Silu, bias=t_h1b[:, m, :])
            ps2 = PSC.tile([2, NT], F32, tag="psA")
            for k in range(2):
                nc.tensor.matmul(ps2[:], t_h2W[:, k, :], h5[:, k, :],
                                 start=(k == 0), stop=(k == 1))
            yst = SK.tile([2, NT], F32, tag="yst")
            nc.vector.tensor_scalar(yst[:], ps2[:], t_h2b[:], None, op0=ALU.add)
            nc.sync.dma_start(yT[:, u:u + NT], yst[:])

    nc.compile()
    return nc


def kernel(**inputs):
    x = np.asarray(inputs["x"], np.float32)
    t = np.asarray(inputs["t"])
    nc = build(inputs)
    in_maps = []
    for c in range(NCORES):
        xs = x[c * BC:(c + 1) * BC]
        xTs = np.ascontiguousarray(xs.reshape(N, 2).T).astype(ml_dtypes.bfloat16)
        ts = t[c * BC:(c + 1) * BC].astype(np.float32)
        tps = np.ascontiguousarray(np.stack([ts, np.ones_like(ts)]))
        in_maps.append({"xT": xTs, "tp": tps})
    res = run_bass_kernel_spmd(nc, in_maps, core_ids=list(range(NCORES)))
    outs = []
    for c in range(NCORES):
        yTs = res.results[c]["yT"]
        outs.append(yTs.T.reshape(BC, 2 * V).astype(np.float32))
    return np.concatenate(outs, 0)



# revision 2
# speedup vs baseline: 1.0203x; 1.0203x over previous
"""DenoiseGAT Trainium2 kernel v2: 8-core data-parallel over polygons.

Per core: 256 polygons x 64 nodes = 16384 nodes, activations feature-major
h[128, 2, N] bf16 with head-major column permutation (head = p//32 for both
halves) so one [128, n] alpha tile serves all 256 features of a head.

Layer pipeline (per 8192-node chunk = 128 polys):
  S: score matmuls (lhsT = W@ablk precomputed) batched 4 subchunks/PSUM bank,
     copied to scT rows, DRAM-transposed to poly-major.
  T: softmax in poly-major [128 polys, (edge, head, v)], alphas written to
     a_dram rows; alpha rows broadcast-DMA'd to af[128, span] tiles.
  C: per 512-node subchunk: proj+skip matmuls; proj copied to SBUF bf16;
     3 alpha-multiplies (DVE TT 2x, +-1 node shifts on the proj AP with
     polygon wrap columns); products accumulated into the skip PSUM via
     identity matmuls on the PE; ELU via max(v+b, min(exp(v+b),1)-1)
     (Act Exp + Act Relu + DVE min/add) written in-place into h.
"""

import numpy as np
import ml_dtypes
from contextlib import ExitStack

import concourse.bass as bass
import concourse.tile as tile
import concourse.tile_utils as tile_utils
from concourse import bacc, mybir
from concourse.bass_utils import run_bass_kernel_spmd

tile_utils.max_sbuf_usage = 208 * 1024

F32 = mybir.dt.float32
BF16 = mybir.dt.bfloat16
ALU = mybir.AluOpType
ACTF = mybir.ActivationFunctionType

NCORES = 8
B, V = 2048, 64
HID, TDIM = 256, 128
BC = B // NCORES            # 256 polygons / core
N = BC * V                  # 16384 nodes / core
NT = 512                    # combine subchunk (8 polys)
SCH = 8192                  # chunk = 128 polys
AFCH = 8192                 # alpha broadcast span (full chunk)

CFG = {
    "projc": "split",   # proj PSUM->SBUF copy engine: act|dve|split (PSUM: no pool)
    "uadds": "pe",      # 'pe': 3 identity-acc matmuls; 'dve1': 2 DVE adds + 1 acc
    "final": "relu",    # 'relu': Act Relu + DVE min + Pool add; 'max': DVE min + max(PSUM)
    "sct": "dve",       # score PSUM->SBUF copy engine: dve|act
    "tembx": "pool",    # temb node-expand copy engine
    "hadd": "pool",     # final mm+rl add engine: pool|dve
}


def _ablk(asrc, atgt):
    NH, FO = asrc.shape
    out = np.zeros((NH * FO, 2 * NH), np.float32)
    for h in range(NH):
        out[h * FO:(h + 1) * FO, h] = asrc[h]
        out[h * FO:(h + 1) * FO, NH + h] = atgt[h]
    return out


def _perm256():
    P = np.zeros(256, np.int64)
    for s in range(256):
        p, half = s % 128, s // 128
        P[s] = (p // 32) * 64 + (p % 32) + 32 * half
    return P


def _bf(a):
    return np.ascontiguousarray(np.asarray(a, np.float32).astype(ml_dtypes.bfloat16))


def _f32(a):
    return np.ascontiguousarray(np.asarray(a, np.float32))


def half3(a):
    """(256, X) host -> (128, 2, X) so tile[:, j, :] == rows 128j:128j+128."""
    a = np.asarray(a)
    return np.ascontiguousarray(a.reshape(2, 128, a.shape[1]).transpose(1, 0, 2))


def bcol(b):
    """(256,) permuted-storage bias -> (128, 2, 1)."""
    return half3(np.asarray(b, np.float32).reshape(-1, 1))


def build(weights):
    nc = bacc.Bacc("TRN2", target_bir_lowering=False, debug=False,
                   enable_asserts=False, num_devices=NCORES)
    w = weights
    P = _perm256()

    def inl(name, arr):
        return nc.inline_tensor(np.ascontiguousarray(arr), name=name).ap()

    # ---- host-side weight prep ----
    half = TDIM // 2
    freqs = np.exp(-np.log(10000.0) * np.arange(half, dtype=np.float32) / (half - 1))
    fr2 = np.stack([np.concatenate([freqs, freqs]),
                    np.concatenate([np.zeros(half, np.float32),
                                    np.full(half, np.pi / 2, np.float32)])])
    ph = np.arange(V, dtype=np.float32) * (2 * np.pi / V)
    posT = np.tile(np.stack([np.sin(ph), np.cos(ph), np.sin(2 * ph), np.cos(2 * ph)]), (1, BC))

    Ws = {i: _f32(w[f"W{i}"]) for i in range(4)}
    sks = {i: _f32(w[f"skip{i}"]) for i in range(3)}
    abs_ = {i: _ablk(_f32(w[f"asrc{i}"]), _f32(w[f"atgt{i}"])) for i in range(4)}
    Was = {i: Ws[i] @ abs_[i] for i in range(4)}

    c_fr2 = inl("fr2", fr2.astype(np.float32))
    c_tW = inl("tW", _f32(w["tW"]))
    c_tb = inl("tb", _f32(w["tb"]).reshape(-1, 1))
    c_posT = inl("posT", _bf(posT))
    c_eye = inl("eye128", _bf(np.eye(128, dtype=np.float32)))

    # L0: per-node part (rows 0:6) and temb part (rows 6:134)
    W0c = Ws[0][:, P]                      # out-cols permuted
    sk0c = sks[0][:, P]
    c_W0f = inl("W0f", _bf(np.concatenate([W0c[:6], sk0c[:6]], 1)))       # (6, 512)
    # temb contributions to L0 (proj part is constant within polygon, so both
    # proj- and skip-temb fold into one post-combine matmul)
    c_Wt0 = inl("Wt0", _bf(W0c[6:] + sk0c[6:]).reshape(128, 1, 256))
    def pad32(a):
        out = np.zeros((a.shape[0], 32), np.float32)
        out[:, :a.shape[1]] = a
        return out

    c_Wa0p = inl("Wa0p", _bf(pad32(Was[0][:6])))                          # (6, 32)
    c_Wa0t = inl("Wa0t", _bf(pad32(Was[0][6:])))                          # (128, 32)
    c_b0 = inl("b0c", bcol(_f32(w["b0"])[P]))

    LW, LWA, LB = {}, {}, {}
    for i in (1, 2):
        Wi = Ws[i][P][:, P]                # in-rows permuted (h storage), out-cols permuted
        si = sks[i][P][:, P]
        LW[i] = inl(f"W{i}f", half3(_bf(np.concatenate([Wi, si], 1))))    # (128,2,1024)
        LWA[i] = inl(f"Wa{i}f", half3(_bf(pad32(Was[i][P]))))             # (128,2,32)
        LB[i] = inl(f"b{i}c", bcol(_f32(w[f"b{i}"])[P]))
    W3p = Ws[3][P][:, P]
    c_W3 = inl("W3f", half3(_bf(W3p)))                                    # (128,2,256)
    c_Wa3 = inl("Wa3f", half3(_bf(pad32(Was[3][P]))))                     # (128,2,32)
    c_b3 = inl("b3c", bcol(_f32(w["b3"])[P]))
    c_h1W = inl("h1Wf", half3(_bf(_f32(w["h1W"])[P])))
    c_h1b = inl("h1bc", bcol(_f32(w["h1b"])))
    c_h2W = inl("h2Wf", half3(_bf(pad32(_f32(w["h2W"])))))
    h2bp = np.zeros((32, 1), np.float32)
    h2bp[0:2, 0] = _f32(w["h2b"])
    c_h2b = inl("h2b32", np.tile(h2bp, (4, 1)))                           # (128,1)

    xT = nc.dram_tensor("xT", [2, N], BF16, kind="ExternalInput").ap()
    tp = nc.dram_tensor("tp", [2, BC], F32, kind="ExternalInput").ap()
    yT = nc.dram_tensor("yT", [2, N], F32, kind="ExternalOutput").ap()

    TT = nc.vector.tensor_tensor
    TS = nc.vector.tensor_scalar
    STT = nc.vector.scalar_tensor_tensor

    def copy_on(engine, out, in_):
        if engine == "pool":
            nc.gpsimd.tensor_copy(out, in_)
        elif engine == "act":
            # per-half activation copies (bias must be scalar per partition)
            nc.scalar.activation(out, in_, ACTF.Copy)
        else:
            nc.vector.tensor_copy(out, in_)

    with tile.TileContext(nc) as tc, ExitStack() as ctx:
        PP = ctx.enter_context(tc.tile_pool(name="pers", bufs=1))
        WP = ctx.enter_context(tc.tile_pool(name="wts", bufs=1))
        DR = ctx.enter_context(tc.tile_pool(name="dram", bufs=1, space="DRAM"))
        PSA = ctx.enter_context(tc.tile_pool(name="psa", bufs=1, space="PSUM"))
        PSK = ctx.enter_context(tc.tile_pool(name="psk", bufs=2, space="PSUM"))
        PSC = ctx.enter_context(tc.tile_pool(name="psc", bufs=2, space="PSUM"))
        SM = ctx.enter_context(tc.tile_pool(name="sm", bufs=2))
        CB = ctx.enter_context(tc.tile_pool(name="cb", bufs=2))
        AF = ctx.enter_context(tc.tile_pool(name="af", bufs=1))
        H0 = ctx.enter_context(tc.tile_pool(name="h0", bufs=1))
        SCT = ctx.enter_context(tc.tile_pool(name="sct", bufs=1))
        TMB = ctx.enter_context(tc.tile_pool(name="tmb", bufs=1))
        YP = ctx.enter_context(tc.tile_pool(name="yp", bufs=1))

        h = PP.tile([128, 2, N], BF16, tag="h")
        tembT = PP.tile([TDIM, BC], F32, tag="tembT")

        def load(c_ap, tag):
            t = WP.tile(list(c_ap.shape), c_ap.dtype, tag=tag)
            nc.sync.dma_start(t[:], c_ap)
            return t

        t_eye = load(c_eye, "eye")

        # ---- temb: SinusoidalPosEmb + Linear + SiLU (baseline pipeline) ----
        t_fr2 = load(c_fr2, "fr2")
        t_tp = load(tp, "tp")
        ps_te = PSC.tile([TDIM, BC], F32, tag="ps_sc", name="ps_te")
        nc.tensor.matmul(ps_te[:], t_fr2[:], t_tp[:], start=True, stop=True)
        te_m = TMB.tile([TDIM, BC], F32, tag="sm1")
        te_q = TMB.tile([TDIM, BC], mybir.dt.int32, tag="sm2")
        nc.vector.tensor_scalar(te_q[:], ps_te[:], float(1.0 / (2 * np.pi)), None, op0=ALU.mult)
        te_qf = TMB.tile([TDIM, BC], F32, tag="sm3")
        nc.vector.tensor_copy(te_qf[:], te_q[:])
        STT(te_m[:], te_qf[:], float(-2 * np.pi), ps_te[:], op0=ALU.mult, op1=ALU.add)
        # fold the remainder into [-pi, pi] regardless of whether the f32->i32
        # cast rounded or truncated
        te_c = TMB.tile([TDIM, BC], F32, tag="sm5")
        nc.vector.tensor_scalar(te_c[:], te_m[:], float(np.pi), None, op0=ALU.is_gt)
        STT(te_m[:], te_c[:], float(-2 * np.pi), te_m[:], op0=ALU.mult, op1=ALU.add)
        te_s = TMB.tile([TDIM, BC], F32, tag="sm4")
        nc.scalar.activation(te_s[:], te_m[:], ACTF.Sin)
        t_tW = load(c_tW, "tW")
        t_tb = load(c_tb, "tb")
        ps_tm = PSC.tile([TDIM, BC], F32, tag="ps_sc", name="ps_tm")
        nc.tensor.matmul(ps_tm[:], t_tW[:], te_s[:], start=True, stop=True)
        nc.scalar.activation(tembT[:], ps_tm[:], ACTF.Silu, bias=t_tb[:])

        # weights
        t_W0f = load(c_W0f, "W0f")
        t_Wt0 = load(c_Wt0, "Wt0")
        t_Wa0p = load(c_Wa0p, "Wa0p")
        t_Wa0t = load(c_Wa0t, "Wa0t")
        t_b = {0: load(c_b0, "b0")}
        t_W = {}
        t_Wa = {}
        for i in (1, 2):
            t_W[i] = load(LW[i], f"W{i}")
            t_Wa[i] = load(LWA[i], f"Wa{i}")
            t_b[i] = load(LB[i], f"b{i}")
        t_W[3] = load(c_W3, "W3")
        t_Wa[3] = load(c_Wa3, "Wa3")
        t_b[3] = load(c_b3, "b3")
        t_h1W = load(c_h1W, "h1W")
        t_h1b = load(c_h1b, "h1b")
        t_h2W = load(c_h2W, "h2W")
        t_h2b = load(c_h2b, "h2b")

        sc_dram = DR.tile([32, N], BF16, tag="sc_dram")
        a_dram = DR.tile([12, N], BF16, tag="a_dram")

        def st_phase(li, ch):
            """Scores + softmax + alpha rows for one 8192-node chunk."""
            first = li == 0
            last = li == 3
            R = 1 if last else 4
            SR = 2 * R
            ER = 3 * R
            spc = 4
            u0 = ch * SCH
            g0 = u0 // V

            if first:
                h0loc = H0.tile([6, SCH], BF16, tag="h0loc")
                nc.sync.dma_start(h0loc[0:2, :], xT[:, u0:u0 + SCH])
                nc.sync.dma_start(h0loc[2:6, :], c_posT[:, u0:u0 + SCH])
                tembX = H0.tile([128, SCH], BF16, tag="tembX")
                src = tembT[:, g0:g0 + SCH // V].unsqueeze(2).to_broadcast(
                    (128, SCH // V, V))
                if CFG["tembx"] == "pool":
                    nc.gpsimd.tensor_copy(tembX[:].rearrange("p (g v) -> p g v", v=V), src)
                else:
                    nc.vector.tensor_copy(tembX[:].rearrange("p (g v) -> p g v", v=V), src)
                h0st[ch] = (h0loc, tembX)

            # scores: 4 subchunks per psum bank at 32-partition offsets
            scT = SCT.tile([128, SCH // spc], BF16, tag="scT")
            for batch in range(SCH // NT // spc):
                ps_sc = PSC.tile([128, NT], F32, tag="ps_sc", name="ps_sc")
                for s4 in range(spc):
                    sub = batch * spc + s4
                    u = u0 + sub * NT
                    rows = slice(32 * s4, 32 * s4 + 32)
                    tp_ = (0, 32 * s4)
                    if first:
                        nc.tensor.matmul(ps_sc[rows, :], t_Wa0p[:],
                                         h0loc[0:6, sub * NT:(sub + 1) * NT],
                                         start=True, stop=False, tile_position=tp_)
                        nc.tensor.matmul(ps_sc[rows, :], t_Wa0t[:],
                                         tembX[:, sub * NT:(sub + 1) * NT],
                                         start=False, stop=True, tile_position=tp_)
                    else:
                        for k in range(2):
                            nc.tensor.matmul(ps_sc[rows, :], t_Wa[li][:, k, :],
                                             h[:, k, u:u + NT],
                                             start=(k == 0), stop=(k == 1),
                                             tile_position=tp_)
                dst = scT[:, batch * NT:(batch + 1) * NT]
                if CFG["sct"] == "dve":
                    nc.vector.tensor_copy(dst, ps_sc[:])
                else:
                    nc.scalar.activation(dst, ps_sc[:], ACTF.Copy)
            for s4 in range(spc):
                scs = scT[32 * s4:32 * s4 + 32, :].rearrange(
                    "r (b w) -> r b w", w=NT)
                scd = sc_dram[0:32, u0:u0 + SCH].rearrange(
                    "r (b s w) -> r b s w", s=spc, w=NT)[:, :, s4, :]
                nc.sync.dma_start(scd, scs)

            # softmax in poly-major
            S = SM.tile([128, SR * V], BF16, tag="Sblk")
            nc.sync.dma_start(
                S[:].rearrange("p (r v) -> p r v", v=V),
                sc_dram[0:SR, u0:u0 + SCH].rearrange("r (p v) -> p r v", v=V))
            E = SM.tile([128, ER * V], BF16, tag="E")
            Sv = S[:].rearrange("p (r v) -> p r v", v=V)
            Ssrc, Stgt = Sv[:, 0:R, :], Sv[:, R:SR, :]
            Ev = E[:].rearrange("p (k r v) -> p k r v", k=3, v=V)
            TT(Ev[:, 0, :, 1:], Ssrc[:, :, :V - 1], Stgt[:, :, 1:], op=ALU.add)
            TT(Ev[:, 0, :, 0:1], Ssrc[:, :, V - 1:], Stgt[:, :, 0:1], op=ALU.add)
            TT(Ev[:, 1, :, :], Ssrc, Stgt, op=ALU.add)
            TT(Ev[:, 2, :, :V - 1], Ssrc[:, :, 1:], Stgt[:, :, :V - 1], op=ALU.add)
            TT(Ev[:, 2, :, V - 1:], Ssrc[:, :, 0:1], Stgt[:, :, V - 1:], op=ALU.add)
            STT(E[:], E[:], 0.2, E[:], op0=ALU.mult, op1=ALU.max)
            EX = SM.tile([128, ER * V], BF16, tag="EX")
            nc.scalar.activation(EX[:], E[:], ACTF.Exp)
            den = SM.tile([128, R * V], F32, tag="den")
            TT(den[:], EX[:, 0:R * V], EX[:, R * V:2 * R * V], op=ALU.add)
            TT(den[:], den[:], EX[:, 2 * R * V:], op=ALU.add)
            rd = SM.tile([128, R * V], F32, tag="rd")
            nc.vector.reciprocal(rd[:], den[:])
            ab = SM.tile([128, ER * V], BF16, tag="E", name="abt")
            for k in range(3):
                TT(ab[:, k * R * V:(k + 1) * R * V],
                   EX[:, k * R * V:(k + 1) * R * V], rd[:], op=ALU.mult)
            nc.sync.dma_start(
                a_dram[0:ER, u0:u0 + SCH].rearrange("j (p v) -> p j v", v=V),
                ab[:].rearrange("p (j v) -> p j v", v=V))

        def c_phase(li, ch):
            """Proj/skip matmuls + alpha combine + activation for one chunk."""
            first = li == 0
            last = li == 3
            R = 1 if last else 4
            kt = 1 if first else 2
            nmb = 2 if last else 4
            u0 = ch * SCH
            h0loc, tembX = h0st[ch] if first else (None, None)

            af3 = []
            for k in range(3):
                afk = AF.tile([128, AFCH], BF16, tag=f"af{k}")
                if R == 1:
                    nc.sync.dma_start(
                        afk[:], a_dram[k:k + 1, u0:u0 + AFCH].to_broadcast((128, AFCH)))
                else:
                    for r in range(R):
                        nc.sync.dma_start(
                            afk[32 * r:32 * r + 32, :],
                            a_dram[k * R + r:k * R + r + 1, u0:u0 + AFCH]
                            .to_broadcast((32, AFCH)))
                af3.append(afk)

            def tail(st):
                ps_k, uts, u = st
                if CFG["uadds"] == "pe":
                    for k in range(3):
                        for m in range(2):
                            nc.tensor.matmul(ps_k[:, m, :], t_eye[:], uts[k][:, m, :],
                                             start=False, stop=(k == 2))
                else:
                    s01 = CB.tile([128, 2, NT], BF16, tag="u0", name="s01")
                    TT(s01[:], uts[0][:], uts[1][:], op=ALU.add)
                    s012 = CB.tile([128, 2, NT], BF16, tag="u1", name="s012")
                    TT(s012[:], s01[:], uts[2][:], op=ALU.add)
                    for m in range(2):
                        nc.tensor.matmul(ps_k[:, m, :], t_eye[:], s012[:, m, :],
                                         start=False, stop=True)
                if not last:
                    Ee = CB.tile([128, 2, NT], BF16, tag="Ee")
                    rl = CB.tile([128, 2, NT], BF16, tag="rl")
                    for m in range(2):
                        nc.scalar.activation(Ee[:, m, :], ps_k[:, m, :], ACTF.Exp,
                                             bias=t_b[li][:, m, :])
                        nc.scalar.activation(rl[:, m, :], ps_k[:, m, :], ACTF.Relu,
                                             bias=t_b[li][:, m, :])
                    mm = CB.tile([128, 2, NT], BF16, tag="mm")
                    TS(mm[:], Ee[:], 1.0, -1.0, op0=ALU.min, op1=ALU.add)
                    if CFG["hadd"] == "pool":
                        nc.gpsimd.tensor_tensor(h[:, :, u:u + NT], mm[:], rl[:],
                                                op=ALU.add)
                    else:
                        TT(h[:, :, u:u + NT], mm[:], rl[:], op=ALU.add)
                else:
                    for m in range(2):
                        nc.scalar.activation(h[:, m, u:u + NT], ps_k[:, m, :],
                                             ACTF.Identity, bias=t_b[li][:, m, :])

            pend = None
            for sub in range(SCH // NT):
                u = u0 + sub * NT
                qo = sub * NT
                ps_p = PSA.tile([128, 2, NT], F32, tag="ps_p")
                ps_k = PSK.tile([128, 2, NT], F32, tag="ps_k")
                for m in range(nmb):
                    dst = ps_p[:, m, :] if m < 2 else ps_k[:, m - 2, :]
                    for k in range(kt):
                        rhs = (h0loc[0:6, sub * NT:(sub + 1) * NT] if first
                               else h[:, k, u:u + NT])
                        lhs = (t_W0f[:, m * 128:(m + 1) * 128] if first
                               else t_W[li][:, k, m * 128:(m + 1) * 128])
                        nc.tensor.matmul(dst, lhs, rhs, start=(k == 0),
                                         stop=(k == kt - 1 and m < 2))
                if first:
                    for m in range(2):
                        nc.tensor.matmul(ps_k[:, m, :], t_Wt0[:, 0, m * 128:(m + 1) * 128],
                                         tembX[:, sub * NT:(sub + 1) * NT],
                                         start=False, stop=False)
                if last:
                    for m in range(2):
                        nc.tensor.matmul(ps_k[:, m, :], t_eye[:], h[:, m, u:u + NT],
                                         start=True, stop=False)

                projc = CB.tile([128, 2, NT], BF16, tag="projc")
                if CFG["projc"] == "act":
                    for m in range(2):
                        nc.scalar.activation(projc[:, m, :], ps_p[:, m, :], ACTF.Copy)
                elif CFG["projc"] == "split":
                    nc.scalar.activation(projc[:, 0, :], ps_p[:, 0, :], ACTF.Copy)
                    nc.vector.tensor_copy(projc[:, 1, :], ps_p[:, 1, :])
                else:
                    nc.vector.tensor_copy(projc[:], ps_p[:])

                pv = projc[:].rearrange("p h (g v) -> p h g v", v=V)
                uts = []
                for k in range(3):
                    ut = CB.tile([128, 2, NT], BF16, tag=f"u{k}", name=f"u{k}")
                    uv = ut[:].rearrange("p h (g v) -> p h g v", v=V)
                    afk = af3[k][:, qo:qo + NT]
                    av2 = afk.unsqueeze(1).to_broadcast((128, 2, NT)) \
                        .rearrange("p h (g v) -> p h g v", v=V)
                    if k == 1:
                        TT(ut[:], afk.unsqueeze(1).to_broadcast((128, 2, NT)),
                           projc[:], op=ALU.mult)
                    elif k == 0:
                        TT(uv[:, :, :, 1:], av2[:, :, :, 1:], pv[:, :, :, :V - 1],
                           op=ALU.mult)
                        TT(uv[:, :, :, 0:1], av2[:, :, :, 0:1], pv[:, :, :, V - 1:],
                           op=ALU.mult)
                    else:
                        TT(uv[:, :, :, :V - 1], av2[:, :, :, :V - 1], pv[:, :, :, 1:],
                           op=ALU.mult)
                        TT(uv[:, :, :, V - 1:], av2[:, :, :, V - 1:], pv[:, :, :, 0:1],
                           op=ALU.mult)
                    uts.append(ut)

                if pend is not None:
                    tail(pend)
                pend = (ps_k, uts, u)
            tail(pend)

        def mlp_chunk(ch):
            pend = None
            cur = {}

            def flush_pend():
                nonlocal pend
                if pend is None:
                    return
                grp, ps_y, ph5, ps4 = pend
                for k in range(2):
                    nc.tensor.matmul(ps_y[32 * ps4:32 * ps4 + 32, :], t_h2W[:, k, :],
                                     ph5[:, k, :], start=(k == 0), stop=(k == 1),
                                     tile_position=(0, 32 * ps4))
                if ps4 == 3:
                    yst = YP.tile([128, NT], F32, tag="yst", name=f"yst{grp}")
                    nc.vector.tensor_scalar(yst[:], ps_y[:], t_h2b[:], None, op0=ALU.add)
                    for q4 in range(4):
                        nc.sync.dma_start(
                            yT[:, (grp * 4 + q4) * NT:(grp * 4 + q4 + 1) * NT],
                            yst[32 * q4:32 * q4 + 2, :])
                pend = None

            for sub in range(16 * ch, 16 * ch + 16):
                grp, s4 = sub // 4, sub % 4
                u = sub * NT
                if s4 == 0:
                    cur[grp] = PSC.tile([128, NT], F32, tag="ps_sc", name=f"psy{grp}")
                ps_h = PSK.tile([128, 2, NT], F32, tag="ps_k", name="ps_h")
                for m in range(2):
                    for k in range(2):
                        nc.tensor.matmul(ps_h[:, m, :],
                                         t_h1W[:, k, m * 128:(m + 1) * 128],
                                         h[:, k, u:u + NT], start=(k == 0),
                                         stop=(k == 1))
                h5 = CB.tile([128, 2, NT], BF16, tag="projc", name="h5")
                for m in range(2):
                    nc.scalar.activation(h5[:, m, :], ps_h[:, m, :], ACTF.Silu,
                                         bias=t_h1b[:, m, :])
                flush_pend()
                pend = (grp, cur[grp], h5, s4)
            flush_pend()

        h0st = {}
        st_phase(0, 0)
        c_phase(0, 0)
        st_phase(0, 1)
        c_phase(0, 1)
        for li in (1, 2, 3):
            st_phase(li, 0)
            c_phase(li, 0)
            st_phase(li, 1)
            c_phase(li, 1)
        mlp_chunk(0)
        mlp_chunk(1)

    nc.compile()
    return nc


def kernel(**inputs):
    x = np.asarray(inputs["x"], np.float32)
    t = np.asarray(inputs["t"])
    nc = build(inputs)
    in_maps = []
    for c in range(NCORES):
        xs = x[c * BC:(c + 1) * BC]
        xTs = np.ascontiguousarray(xs.reshape(N, 2).T).astype(ml_dtypes.bfloat16)
        ts = t[c * BC:(c + 1) * BC].astype(np.float32)
        tps = np.ascontiguousarray(np.stack([ts, np.ones_like(ts)]))
        in_maps.append({"xT": xTs, "tp": tps})
    res = run_bass_kernel_spmd(nc, in_maps, core_ids=list(range(NCORES)))
    outs = []
    for c in range(NCORES):
        yTs = res.results[c]["yT"]
        outs.append(yTs.T.reshape(BC, 2 * V).astype(np.float32))
    return np.concatenate(outs, 0)


# revision 3
# speedup vs baseline: 1.1138x; 1.0917x over previous
"""DenoiseGAT Trainium2 kernel v2: 8-core data-parallel over polygons.

Per core: 256 polygons x 64 nodes = 16384 nodes, activations feature-major
h[128, 2, N] bf16 with head-major column permutation (head = p//32 for both
halves) so one [128, n] alpha tile serves all 256 features of a head.

Layer pipeline (per 8192-node chunk = 128 polys):
  S: score matmuls (lhsT = W@ablk precomputed) batched 4 subchunks/PSUM bank,
     copied to scT rows, DRAM-transposed to poly-major.
  T: softmax in poly-major [128 polys, (edge, head, v)], alphas written to
     a_dram rows; alpha rows broadcast-DMA'd to af[128, span] tiles.
  C: per 512-node subchunk: proj+skip matmuls; proj copied to SBUF bf16;
     3 alpha-multiplies (DVE TT 2x, +-1 node shifts on the proj AP with
     polygon wrap columns); products accumulated into the skip PSUM via
     identity matmuls on the PE; ELU via max(v+b, min(exp(v+b),1)-1)
     (Act Exp + Act Relu + DVE min/add) written in-place into h.
"""

import numpy as np
import ml_dtypes
from contextlib import ExitStack

import concourse.bass as bass
import concourse.tile as tile
import concourse.tile_utils as tile_utils
from concourse import bacc, mybir
from concourse.bass_utils import run_bass_kernel_spmd

tile_utils.max_sbuf_usage = 208 * 1024

F32 = mybir.dt.float32
BF16 = mybir.dt.bfloat16
ALU = mybir.AluOpType
ACTF = mybir.ActivationFunctionType

NCORES = 8
B, V = 2048, 64
HID, TDIM = 256, 128
BC = B // NCORES            # 256 polygons / core
N = BC * V                  # 16384 nodes / core
NT = 512                    # combine subchunk (8 polys)
SCH = 8192                  # chunk = 128 polys
AFCH = 8192                 # alpha broadcast span (full chunk)

CFG = {
    "projc": "split",   # proj PSUM->SBUF copy engine: act|dve|split (PSUM: no pool)
    "uadds": "pe",      # 'pe': 3 identity-acc matmuls; 'dve1': 2 DVE adds + 1 acc
    "final": "relu",    # 'relu': Act Relu + DVE min + Pool add; 'max': DVE min + max(PSUM)
    "sct": "dve",       # score PSUM->SBUF copy engine: dve|act
    "tembx": "pool",    # temb node-expand copy engine
    "hadd": "pool",     # final mm+rl add engine: pool|dve
}


def _ablk(asrc, atgt):
    NH, FO = asrc.shape
    out = np.zeros((NH * FO, 2 * NH), np.float32)
    for h in range(NH):
        out[h * FO:(h + 1) * FO, h] = asrc[h]
        out[h * FO:(h + 1) * FO, NH + h] = atgt[h]
    return out


def _perm256():
    P = np.zeros(256, np.int64)
    for s in range(256):
        p, half = s % 128, s // 128
        P[s] = (p // 32) * 64 + (p % 32) + 32 * half
    return P


def _bf(a):
    return np.ascontiguousarray(np.asarray(a, np.float32).astype(ml_dtypes.bfloat16))


def _f32(a):
    return np.ascontiguousarray(np.asarray(a, np.float32))


def half3(a):
    """(256, X) host -> (128, 2, X) so tile[:, j, :] == rows 128j:128j+128."""
    a = np.asarray(a)
    return np.ascontiguousarray(a.reshape(2, 128, a.shape[1]).transpose(1, 0, 2))


def bcol(b):
    """(256,) permuted-storage bias -> (128, 2, 1)."""
    return half3(np.asarray(b, np.float32).reshape(-1, 1))


def build(weights):
    nc = bacc.Bacc("TRN2", target_bir_lowering=False, debug=False,
                   enable_asserts=False, num_devices=NCORES)
    w = weights
    P = _perm256()

    def inl(name, arr):
        return nc.inline_tensor(np.ascontiguousarray(arr), name=name).ap()

    # ---- host-side weight prep ----
    half = TDIM // 2
    freqs = np.exp(-np.log(10000.0) * np.arange(half, dtype=np.float32) / (half - 1))
    fr2 = np.stack([np.concatenate([freqs, freqs]),
                    np.concatenate([np.zeros(half, np.float32),
                                    np.full(half, np.pi / 2, np.float32)])])
    ph = np.arange(V, dtype=np.float32) * (2 * np.pi / V)
    posT = np.tile(np.stack([np.sin(ph), np.cos(ph), np.sin(2 * ph), np.cos(2 * ph)]), (1, BC))

    Ws = {i: _f32(w[f"W{i}"]) for i in range(4)}
    sks = {i: _f32(w[f"skip{i}"]) for i in range(3)}
    abs_ = {i: _ablk(_f32(w[f"asrc{i}"]), _f32(w[f"atgt{i}"])) for i in range(4)}
    Was = {i: Ws[i] @ abs_[i] for i in range(4)}

    c_fr2 = inl("fr2", fr2.astype(np.float32))
    c_tW = inl("tW", _f32(w["tW"]))
    c_tb = inl("tb", _f32(w["tb"]).reshape(-1, 1))
    c_posT = inl("posT", _bf(posT))
    c_eye = inl("eye128", _bf(np.eye(128, dtype=np.float32)))

    # L0: per-node part (rows 0:6) and temb part (rows 6:134)
    W0c = Ws[0][:, P]                      # out-cols permuted
    sk0c = sks[0][:, P]
    c_W0f = inl("W0f", _bf(np.concatenate([W0c[:6], sk0c[:6]], 1)))       # (6, 512)
    # temb contributions to L0 (proj part is constant within polygon, so both
    # proj- and skip-temb fold into one post-combine matmul)
    c_Wt0 = inl("Wt0", _bf(W0c[6:] + sk0c[6:]).reshape(128, 1, 256))
    def pad32(a):
        out = np.zeros((a.shape[0], 32), np.float32)
        out[:, :a.shape[1]] = a
        return out

    c_Wa0p = inl("Wa0p", _bf(pad32(Was[0][:6])))                          # (6, 32)
    c_Wa0t = inl("Wa0t", _bf(pad32(Was[0][6:])))                          # (128, 32)
    c_b0 = inl("b0c", bcol(_f32(w["b0"])[P]))

    LW, LWA, LB = {}, {}, {}
    for i in (1, 2):
        Wi = Ws[i][P][:, P]                # in-rows permuted (h storage), out-cols permuted
        si = sks[i][P][:, P]
        LW[i] = inl(f"W{i}f", half3(_bf(np.concatenate([Wi, si], 1))))    # (128,2,1024)
        LWA[i] = inl(f"Wa{i}f", half3(_bf(pad32(Was[i][P]))))             # (128,2,32)
        LB[i] = inl(f"b{i}c", bcol(_f32(w[f"b{i}"])[P]))
    W3p = Ws[3][P][:, P]
    c_W3 = inl("W3f", half3(_bf(W3p)))                                    # (128,2,256)
    c_Wa3 = inl("Wa3f", half3(_bf(pad32(Was[3][P]))))                     # (128,2,32)
    c_b3 = inl("b3c", bcol(_f32(w["b3"])[P]))
    c_h1W = inl("h1Wf", half3(_bf(_f32(w["h1W"])[P])))
    c_h1b = inl("h1bc", bcol(_f32(w["h1b"])))
    c_h2W = inl("h2Wf", half3(_bf(pad32(_f32(w["h2W"])))))
    h2bp = np.zeros((32, 1), np.float32)
    h2bp[0:2, 0] = _f32(w["h2b"])
    c_h2b = inl("h2b32", np.tile(h2bp, (4, 1)))                           # (128,1)

    xT = nc.dram_tensor("xT", [2, N], BF16, kind="ExternalInput").ap()
    tp = nc.dram_tensor("tp", [2, BC], F32, kind="ExternalInput").ap()
    yT = nc.dram_tensor("yT", [2, N], F32, kind="ExternalOutput").ap()

    TT = nc.vector.tensor_tensor
    TS = nc.vector.tensor_scalar
    STT = nc.vector.scalar_tensor_tensor

    def copy_on(engine, out, in_):
        if engine == "pool":
            nc.gpsimd.tensor_copy(out, in_)
        elif engine == "act":
            # per-half activation copies (bias must be scalar per partition)
            nc.scalar.activation(out, in_, ACTF.Copy)
        else:
            nc.vector.tensor_copy(out, in_)

    with tile.TileContext(nc) as tc, ExitStack() as ctx:
        PP = ctx.enter_context(tc.tile_pool(name="pers", bufs=1))
        WP = ctx.enter_context(tc.tile_pool(name="wts", bufs=1))
        DR = ctx.enter_context(tc.tile_pool(name="dram", bufs=1, space="DRAM"))
        PSA = ctx.enter_context(tc.tile_pool(name="psa", bufs=1, space="PSUM"))
        PSK = ctx.enter_context(tc.tile_pool(name="psk", bufs=2, space="PSUM"))
        PSC = ctx.enter_context(tc.tile_pool(name="psc", bufs=2, space="PSUM"))
        SM = ctx.enter_context(tc.tile_pool(name="sm", bufs=2))
        CB = ctx.enter_context(tc.tile_pool(name="cb", bufs=2))
        AF = ctx.enter_context(tc.tile_pool(name="af", bufs=1))
        H0 = ctx.enter_context(tc.tile_pool(name="h0", bufs=1))
        SCT = ctx.enter_context(tc.tile_pool(name="sct", bufs=1))
        TMB = ctx.enter_context(tc.tile_pool(name="tmb", bufs=1))
        YP = ctx.enter_context(tc.tile_pool(name="yp", bufs=1))

        h = PP.tile([128, 2, N], BF16, tag="h")
        tembT = PP.tile([TDIM, BC], F32, tag="tembT")

        def load(c_ap, tag):
            t = WP.tile(list(c_ap.shape), c_ap.dtype, tag=tag)
            nc.sync.dma_start(t[:], c_ap)
            return t

        t_eye = load(c_eye, "eye")

        # ---- temb: SinusoidalPosEmb + Linear + SiLU (baseline pipeline) ----
        t_fr2 = load(c_fr2, "fr2")
        t_tp = load(tp, "tp")
        ps_te = PSC.tile([TDIM, BC], F32, tag="ps_sc", name="ps_te")
        nc.tensor.matmul(ps_te[:], t_fr2[:], t_tp[:], start=True, stop=True)
        te_m = TMB.tile([TDIM, BC], F32, tag="sm1")
        te_q = TMB.tile([TDIM, BC], mybir.dt.int32, tag="sm2")
        nc.vector.tensor_scalar(te_q[:], ps_te[:], float(1.0 / (2 * np.pi)), None, op0=ALU.mult)
        te_qf = TMB.tile([TDIM, BC], F32, tag="sm3")
        nc.vector.tensor_copy(te_qf[:], te_q[:])
        STT(te_m[:], te_qf[:], float(-2 * np.pi), ps_te[:], op0=ALU.mult, op1=ALU.add)
        # fold the remainder into [-pi, pi] regardless of whether the f32->i32
        # cast rounded or truncated
        te_c = TMB.tile([TDIM, BC], F32, tag="sm5")
        nc.vector.tensor_scalar(te_c[:], te_m[:], float(np.pi), None, op0=ALU.is_gt)
        STT(te_m[:], te_c[:], float(-2 * np.pi), te_m[:], op0=ALU.mult, op1=ALU.add)
        te_s = TMB.tile([TDIM, BC], F32, tag="sm4")
        nc.scalar.activation(te_s[:], te_m[:], ACTF.Sin)
        t_tW = load(c_tW, "tW")
        t_tb = load(c_tb, "tb")
        ps_tm = PSC.tile([TDIM, BC], F32, tag="ps_sc", name="ps_tm")
        nc.tensor.matmul(ps_tm[:], t_tW[:], te_s[:], start=True, stop=True)
        nc.scalar.activation(tembT[:], ps_tm[:], ACTF.Silu, bias=t_tb[:])

        # weights
        t_W0f = load(c_W0f, "W0f")
        t_Wt0 = load(c_Wt0, "Wt0")
        t_Wa0p = load(c_Wa0p, "Wa0p")
        t_Wa0t = load(c_Wa0t, "Wa0t")
        t_b = {0: load(c_b0, "b0")}
        t_W = {}
        t_Wa = {}
        for i in (1, 2):
            t_W[i] = load(LW[i], f"W{i}")
            t_Wa[i] = load(LWA[i], f"Wa{i}")
            t_b[i] = load(LB[i], f"b{i}")
        t_W[3] = load(c_W3, "W3")
        t_Wa[3] = load(c_Wa3, "Wa3")
        t_b[3] = load(c_b3, "b3")
        t_h1W = load(c_h1W, "h1W")
        t_h1b = load(c_h1b, "h1b")
        t_h2W = load(c_h2W, "h2W")
        t_h2b = load(c_h2b, "h2b")

        sc_dram = DR.tile([32, N], BF16, tag="sc_dram")
        a_dram = DR.tile([12, N], BF16, tag="a_dram")

        def st_phase(li, ch):
            """Scores + softmax + alpha rows for one 8192-node chunk."""
            first = li == 0
            last = li == 3
            R = 1 if last else 4
            SR = 2 * R
            ER = 3 * R
            spc = 4
            u0 = ch * SCH
            g0 = u0 // V

            if first:
                h0loc = H0.tile([6, SCH], BF16, tag="h0loc")
                nc.sync.dma_start(h0loc[0:2, :], xT[:, u0:u0 + SCH])
                nc.sync.dma_start(h0loc[2:6, :], c_posT[:, u0:u0 + SCH])
                tembX = H0.tile([128, SCH], BF16, tag="tembX")
                src = tembT[:, g0:g0 + SCH // V].unsqueeze(2).to_broadcast(
                    (128, SCH // V, V))
                if CFG["tembx"] == "pool":
                    nc.gpsimd.tensor_copy(tembX[:].rearrange("p (g v) -> p g v", v=V), src)
                else:
                    nc.vector.tensor_copy(tembX[:].rearrange("p (g v) -> p g v", v=V), src)
                h0st[ch] = (h0loc, tembX)

            # scores: 4 subchunks per psum bank at 32-partition offsets
            scT = SCT.tile([128, SCH // spc], BF16, tag="scT")
            for batch in range(SCH // NT // spc):
                ps_sc = PSC.tile([128, NT], F32, tag="ps_sc", name="ps_sc")
                for s4 in range(spc):
                    sub = batch * spc + s4
                    u = u0 + sub * NT
                    rows = slice(32 * s4, 32 * s4 + 32)
                    tp_ = (0, 32 * s4)
                    if first:
                        nc.tensor.matmul(ps_sc[rows, :], t_Wa0p[:],
                                         h0loc[0:6, sub * NT:(sub + 1) * NT],
                                         start=True, stop=False, tile_position=tp_)
                        nc.tensor.matmul(ps_sc[rows, :], t_Wa0t[:],
                                         tembX[:, sub * NT:(sub + 1) * NT],
                                         start=False, stop=True, tile_position=tp_)
                    else:
                        for k in range(2):
                            nc.tensor.matmul(ps_sc[rows, :], t_Wa[li][:, k, :],
                                             h[:, k, u:u + NT],
                                             start=(k == 0), stop=(k == 1),
                                             tile_position=tp_)
                dst = scT[:, batch * NT:(batch + 1) * NT]
                if CFG["sct"] == "dve":
                    nc.vector.tensor_copy(dst, ps_sc[:])
                else:
                    nc.scalar.activation(dst, ps_sc[:], ACTF.Copy)
            for s4 in range(spc):
                scs = scT[32 * s4:32 * s4 + 32, :].rearrange(
                    "r (b w) -> r b w", w=NT)
                scd = sc_dram[0:32, u0:u0 + SCH].rearrange(
                    "r (b s w) -> r b s w", s=spc, w=NT)[:, :, s4, :]
                nc.sync.dma_start(scd, scs)

            # softmax in poly-major
            S = SM.tile([128, SR * V], BF16, tag="Sblk")
            nc.sync.dma_start(
                S[:].rearrange("p (r v) -> p r v", v=V),
                sc_dram[0:SR, u0:u0 + SCH].rearrange("r (p v) -> p r v", v=V))
            E = SM.tile([128, ER * V], BF16, tag="E")
            Sv = S[:].rearrange("p (r v) -> p r v", v=V)
            Ssrc, Stgt = Sv[:, 0:R, :], Sv[:, R:SR, :]
            Ev = E[:].rearrange("p (k r v) -> p k r v", k=3, v=V)
            TT(Ev[:, 0, :, 1:], Ssrc[:, :, :V - 1], Stgt[:, :, 1:], op=ALU.add)
            TT(Ev[:, 0, :, 0:1], Ssrc[:, :, V - 1:], Stgt[:, :, 0:1], op=ALU.add)
            TT(Ev[:, 1, :, :], Ssrc, Stgt, op=ALU.add)
            TT(Ev[:, 2, :, :V - 1], Ssrc[:, :, 1:], Stgt[:, :, :V - 1], op=ALU.add)
            TT(Ev[:, 2, :, V - 1:], Ssrc[:, :, 0:1], Stgt[:, :, V - 1:], op=ALU.add)
            STT(E[:], E[:], 0.2, E[:], op0=ALU.mult, op1=ALU.max)
            EX = SM.tile([128, ER * V], BF16, tag="EX")
            nc.scalar.activation(EX[:], E[:], ACTF.Exp)
            den = SM.tile([128, R * V], F32, tag="den")
            TT(den[:], EX[:, 0:R * V], EX[:, R * V:2 * R * V], op=ALU.add)
            TT(den[:], den[:], EX[:, 2 * R * V:], op=ALU.add)
            rd = SM.tile([128, R * V], F32, tag="rd")
            nc.vector.reciprocal(rd[:], den[:])
            ab = SM.tile([128, ER * V], BF16, tag="E", name="abt")
            for k in range(3):
                TT(ab[:, k * R * V:(k + 1) * R * V],
                   EX[:, k * R * V:(k + 1) * R * V], rd[:], op=ALU.mult)
            nc.sync.dma_start(
                a_dram[0:ER, u0:u0 + SCH].rearrange("j (p v) -> p j v", v=V),
                ab[:].rearrange("p (j v) -> p j v", v=V))

        def c_phase(li, ch):
            """Proj/skip matmuls + alpha combine + activation for one chunk."""
            first = li == 0
            last = li == 3
            R = 1 if last else 4
            kt = 1 if first else 2
            nmb = 2 if last else 4
            u0 = ch * SCH
            h0loc, tembX = h0st[ch] if first else (None, None)

            af3 = []
            for k in range(3):
                afk = AF.tile([128, AFCH], BF16, tag=f"af{k}")
                if R == 1:
                    nc.sync.dma_start(
                        afk[:], a_dram[k:k + 1, u0:u0 + AFCH].to_broadcast((128, AFCH)))
                else:
                    for r in range(R):
                        nc.sync.dma_start(
                            afk[32 * r:32 * r + 32, :],
                            a_dram[k * R + r:k * R + r + 1, u0:u0 + AFCH]
                            .to_broadcast((32, AFCH)))
                af3.append(afk)

            def tail(st):
                ps_k, uts, u = st
                if CFG["uadds"] == "pe":
                    for k in range(3):
                        for m in range(2):
                            nc.tensor.matmul(ps_k[:, m, :], t_eye[:], uts[k][:, m, :],
                                             start=False, stop=(k == 2))
                else:
                    s01 = CB.tile([128, 2, NT], BF16, tag="u0", name="s01")
                    TT(s01[:], uts[0][:], uts[1][:], op=ALU.add)
                    s012 = CB.tile([128, 2, NT], BF16, tag="u1", name="s012")
                    TT(s012[:], s01[:], uts[2][:], op=ALU.add)
                    for m in range(2):
                        nc.tensor.matmul(ps_k[:, m, :], t_eye[:], s012[:, m, :],
                                         start=False, stop=True)
                if not last:
                    Ee = CB.tile([128, 2, NT], BF16, tag="Ee")
                    rl = CB.tile([128, 2, NT], BF16, tag="rl")
                    for m in range(2):
                        nc.scalar.activation(Ee[:, m, :], ps_k[:, m, :], ACTF.Exp,
                                             bias=t_b[li][:, m, :])
                        nc.scalar.activation(rl[:, m, :], ps_k[:, m, :], ACTF.Relu,
                                             bias=t_b[li][:, m, :])
                    mm = CB.tile([128, 2, NT], BF16, tag="mm")
                    TS(mm[:], Ee[:], 1.0, -1.0, op0=ALU.min, op1=ALU.add)
                    if CFG["hadd"] == "pool":
                        nc.gpsimd.tensor_tensor(h[:, :, u:u + NT], mm[:], rl[:],
                                                op=ALU.add)
                    else:
                        TT(h[:, :, u:u + NT], mm[:], rl[:], op=ALU.add)
                else:
                    for m in range(2):
                        nc.scalar.activation(h[:, m, u:u + NT], ps_k[:, m, :],
                                             ACTF.Identity, bias=t_b[li][:, m, :])

            pend = None
            for sub in range(SCH // NT):
                u = u0 + sub * NT
                qo = sub * NT
                ps_p = PSA.tile([128, 2, NT], F32, tag="ps_p")
                ps_k = PSK.tile([128, 2, NT], F32, tag="ps_k")
                for m in range(nmb):
                    dst = ps_p[:, m, :] if m < 2 else ps_k[:, m - 2, :]
                    for k in range(kt):
                        rhs = (h0loc[0:6, sub * NT:(sub + 1) * NT] if first
                               else h[:, k, u:u + NT])
                        lhs = (t_W0f[:, m * 128:(m + 1) * 128] if first
                               else t_W[li][:, k, m * 128:(m + 1) * 128])
                        nc.tensor.matmul(dst, lhs, rhs, start=(k == 0),
                                         stop=(k == kt - 1 and m < 2))
                if first:
                    for m in range(2):
                        nc.tensor.matmul(ps_k[:, m, :], t_Wt0[:, 0, m * 128:(m + 1) * 128],
                                         tembX[:, sub * NT:(sub + 1) * NT],
                                         start=False, stop=False)
                if last:
                    for m in range(2):
                        nc.tensor.matmul(ps_k[:, m, :], t_eye[:], h[:, m, u:u + NT],
                                         start=True, stop=False)

                projc = CB.tile([128, 2, NT], BF16, tag="projc")
                if CFG["projc"] == "act":
                    for m in range(2):
                        nc.scalar.activation(projc[:, m, :], ps_p[:, m, :], ACTF.Copy)
                elif CFG["projc"] == "split":
                    nc.scalar.activation(projc[:, 0, :], ps_p[:, 0, :], ACTF.Copy)
                    nc.vector.tensor_copy(projc[:, 1, :], ps_p[:, 1, :])
                else:
                    nc.vector.tensor_copy(projc[:], ps_p[:])

                pv = projc[:].rearrange("p h (g v) -> p h g v", v=V)
                uts = []
                for k in range(3):
                    ut = CB.tile([128, 2, NT], BF16, tag=f"u{k}", name=f"u{k}")
                    uv = ut[:].rearrange("p h (g v) -> p h g v", v=V)
                    afk = af3[k][:, qo:qo + NT]
                    av2 = afk.unsqueeze(1).to_broadcast((128, 2, NT)) \
                        .rearrange("p h (g v) -> p h g v", v=V)
                    if k == 1:
                        TT(ut[:], afk.unsqueeze(1).to_broadcast((128, 2, NT)),
                           projc[:], op=ALU.mult)
                    elif k == 0:
                        TT(uv[:, :, :, 1:], av2[:, :, :, 1:], pv[:, :, :, :V - 1],
                           op=ALU.mult)
                        TT(uv[:, :, :, 0:1], av2[:, :, :, 0:1], pv[:, :, :, V - 1:],
                           op=ALU.mult)
                    else:
                        TT(uv[:, :, :, :V - 1], av2[:, :, :, :V - 1], pv[:, :, :, 1:],
                           op=ALU.mult)
                        TT(uv[:, :, :, V - 1:], av2[:, :, :, V - 1:], pv[:, :, :, 0:1],
                           op=ALU.mult)
                    uts.append(ut)

                if pend is not None:
                    tail(pend)
                pend = (ps_k, uts, u)
            tail(pend)

        def mlp_chunk(ch):
            pend = None
            cur = {}

            def flush_pend():
                nonlocal pend
                if pend is None:
                    return
                grp, ps_y, ph5, ps4 = pend
                for k in range(2):
                    nc.tensor.matmul(ps_y[32 * ps4:32 * ps4 + 32, :], t_h2W[:, k, :],
                                     ph5[:, k, :], start=(k == 0), stop=(k == 1),
                                     tile_position=(0, 32 * ps4))
                if ps4 == 3:
                    yst = YP.tile([128, NT], F32, tag="yst", name=f"yst{grp}")
                    nc.vector.tensor_scalar(yst[:], ps_y[:], t_h2b[:], None, op0=ALU.add)
                    for q4 in range(4):
                        nc.sync.dma_start(
                            yT[:, (grp * 4 + q4) * NT:(grp * 4 + q4 + 1) * NT],
                            yst[32 * q4:32 * q4 + 2, :])
                pend = None

            for sub in range(16 * ch, 16 * ch + 16):
                grp, s4 = sub // 4, sub % 4
                u = sub * NT
                if s4 == 0:
                    cur[grp] = PSC.tile([128, NT], F32, tag="ps_sc", name=f"psy{grp}")
                ps_h = PSK.tile([128, 2, NT], F32, tag="ps_k", name="ps_h")
                for m in range(2):
                    for k in range(2):
                        nc.tensor.matmul(ps_h[:, m, :],
                                         t_h1W[:, k, m * 128:(m + 1) * 128],
                                         h[:, k, u:u + NT], start=(k == 0),
                                         stop=(k == 1))
                h5 = CB.tile([128, 2, NT], BF16, tag="projc", name="h5")
                for m in range(2):
                    nc.scalar.activation(h5[:, m, :], ps_h[:, m, :], ACTF.Silu,
                                         bias=t_h1b[:, m, :])
                flush_pend()
                pend = (grp, cur[grp], h5, s4)
            flush_pend()

        h0st = {}
        st_phase(0, 0)
        c_phase(0, 0)
        st_phase(0, 1)
        c_phase(0, 1)
        for li in (1, 2):
            st_phase(li, 0)
            c_phase(li, 0)
            st_phase(li, 1)
            c_phase(li, 1)
        st_phase(3, 0)
        c_phase(3, 0)
        st_phase(3, 1)
        mlp_chunk(0)
        c_phase(3, 1)
        mlp_chunk(1)

    nc.compile()
    return nc


def kernel(**inputs):
    x = np.asarray(inputs["x"], np.float32)
    t = np.asarray(inputs["t"])
    nc = build(inputs)
    in_maps = []
    for c in range(NCORES):
        xs = x[c * BC:(c + 1) * BC]
        xTs = np.ascontiguousarray(xs.reshape(N, 2).T).astype(ml_dtypes.bfloat16)
        ts = t[c * BC:(c + 1) * BC].astype(np.float32)
        tps = np.ascontiguousarray(np.stack([ts, np.ones_like(ts)]))
        in_maps.append({"xT": xTs, "tp": tps})
    res = run_bass_kernel_spmd(nc, in_maps, core_ids=list(range(NCORES)))
    outs = []
    for c in range(NCORES):
        yTs = res.results[c]["yT"]
        outs.append(yTs.T.reshape(BC, 2 * V).astype(np.float32))
    return np.concatenate(outs, 0)
